# revision 33
# baseline (speedup 1.0000x reference)
"""Multi-head attention (B=2,T=2048,D=1024,H=16,DK=64, causal, RoPE) on 8 TRN2 cores.

Sharding: data-parallel over batch (2) x tensor-parallel over heads (16 -> 4 per
core). core = 4*b + g handles batch b, heads [4g..4g+3]. RoPE tables replicated.
Host pre-transposes x and the projection weights, and permutes the q/k head dims
into [x1(32); x2(32)] blocks per head so RoPE is pure elementwise work on chip.
Each core returns a partial output projection; the host sums the 4 head-group
partials per batch and adds the output bias.

Fused schedule: the kernel is ONE softmax-paced score/exp stream (positions
(chunk j, head-pair dt) in causal order) with ALL other PE work -- later
chunks' q/k/v projections, PV chains, epilogues, out-projection -- interleaved
as cost-budgeted fillers between score pairs.  This overlaps the ACT exp
stream (~80us) with the projection phase instead of serializing the two, and
keeps the PE dense (HAM stays warm).  PSUM is hand-placed in one 8-bank tile:
  banks 0,1: qp(dt) -> kp(dt) -> v chains      (projection home, time-muxed)
  banks 2,3: ct(hh) PV accumulators / po units (attention-consumer home)
  banks 4-7: the two [128,2,512] score tiles   (metronome, double-buffered)
(chunk 0 runs before any scores exist and uses banks 2,3 for kp so its
rope never stalls the PE).
"""

import sys

for _p in ("/opt/trn_rl_repo", "/root/.axon_site/_ro/trn_rl_repo"):
    if _p not in sys.path:
        sys.path.append(_p)

import numpy as np

from concourse import bacc, tile, mybir
import concourse.bass as bass
from concourse.bass2jax import _bass_exec_p, install_neuronx_cc_hook

B, T, D, H, DK = 2, 2048, 1024, 16, 64
G = 4          # heads per core
DSH = G * DK   # 256 sharded head dims per core
NCORES = 8
KT = D // 128  # 8 contraction tiles for projections
NTT = T // 128  # 16 row tiles
NCH = T // 512  # 4 column chunks
F32 = mybir.dt.float32
F32R = mybir.dt.float32r
BF16 = mybir.dt.bfloat16

_CACHE = {}


def _build_bass():
    nc = bacc.Bacc("TRN2", target_bir_lowering=False, debug=False)

    xT = nc.dram_tensor("xT", [D, T], BF16, kind="ExternalInput").ap()
    wqT = nc.dram_tensor("wqT", [128, KT * DSH], BF16, kind="ExternalInput").ap()
    wkT = nc.dram_tensor("wkT", [128, KT * DSH], BF16, kind="ExternalInput").ap()
    wvT = nc.dram_tensor("wvT", [128, KT * DSH], BF16, kind="ExternalInput").ap()
    woT = nc.dram_tensor("woT", [128, 2 * D], BF16, kind="ExternalInput").ap()
    bqk = nc.dram_tensor("bqk", [128, 4], F32, kind="ExternalInput").ap()
    bv = nc.dram_tensor("bv", [1, DSH], F32, kind="ExternalInput").ap()
    cc = nc.dram_tensor("cc", [128, T], BF16, kind="ExternalInput").ap()
    ss = nc.dram_tensor("ss", [128, T], BF16, kind="ExternalInput").ap()
    m01 = nc.dram_tensor("m01", [128, 128], BF16, kind="ExternalInput").ap()
    ones = nc.dram_tensor("ones", [1, 128], F32, kind="ExternalInput").ap()
    out = nc.dram_tensor("out", [T, D], BF16, kind="ExternalOutput").ap()

    with tile.TileContext(nc) as tc:
        with (
            tc.tile_pool(name="const", bufs=1) as const,
            tc.tile_pool(name="persist", bufs=1) as persist,
            tc.tile_pool(name="rope", bufs=2) as ropep,
            tc.tile_pool(name="attn", bufs=2) as attnp,
            tc.tile_pool(name="epi", bufs=2) as epip,
            tc.tile_pool(name="ps", bufs=1, space="PSUM") as psp,
        ):
            # ---- the one 8-bank PSUM tile; every accumulator is a view ----
            PS = psp.tile([128, 8, 512], F32)
            qp = [PS[:, dt, :] for dt in range(2)]           # banks 0,1
            kp01 = [PS[:, dt, :] for dt in range(2)]         # banks 0,1 (chunks>=1)
            kp23 = [PS[:, 2 + dt, :] for dt in range(2)]     # banks 2,3 (chunk 0)
            vp2 = [PS[:, i, 0:256] for i in range(2)]        # banks 0,1 alternating
            cts = [PS[0:65, 2 + hh, :] for hh in range(2)]   # banks 2,3
            pos2 = [PS[:, 2 + i, :] for i in range(2)]       # banks 2,3 (po units)
            scs = [PS[:, 4 + 2 * hh : 6 + 2 * hh, :] for hh in range(2)]  # banks 4-7

            # ---- resident tensors; DMAs issued in consumption order ----
            # sync + scalar are the two HWDGE queues; gpsimd DMAs ride the
            # software DGE.  Full-row transfers (4KB/partition-line) keep the
            # descriptor count minimal -- per-dma sequencer time (~0.6us) is
            # the binding constraint, not HBM bandwidth.
            wq_sb = const.tile([128, KT, DSH], BF16)
            wk_sb = const.tile([128, KT, DSH], BF16)
            wv_sb = const.tile([128, KT, DSH], BF16)
            hw = KT // 2 * DSH
            xk = [const.tile([128, T], BF16, name=f"xk{_k}") for _k in range(KT)]
            cc_sb = const.tile([128, T], BF16)
            ss_sb = const.tile([128, T], BF16)
            bqk_sb = const.tile([128, 4], F32)
            bv_sb = const.tile([1, DSH], F32)
            m01_sb = const.tile([128, 128], BF16)
            wo_sb = const.tile([128, 2, D], BF16)
            # DMA issue order == prologue consumption order so chunk-0
            # projections run dense from ~3us behind the stream head.
            # x rows are split in column halves: the first halves (chunks
            # 0+1) arrive at 2x the row cadence.
            # the two HWDGE queues drain concurrently at ~equal shares of the
            # HBM pool, so the first matmul's inputs (wq k-tiles 0-3 + x
            # chunk-0 halves) are split across BOTH queues instead of
            # serializing behind one 0.75MB wq transfer on sync
            wqf = wq_sb.rearrange("p k n -> p (k n)")
            nc.sync.dma_start(out=wqf[:, 0 : hw // 2], in_=wqT[:, 0 : hw // 2])
            nc.scalar.dma_start(out=wqf[:, hw // 2 : hw], in_=wqT[:, hw // 2 : hw])
            nc.sync.dma_start(out=bqk_sb, in_=bqk)
            nc.sync.dma_start(out=bv_sb, in_=bv)
            for half in range(2):
                csl = slice(1024 * half, 1024 * half + 1024)
                for k in range(KT):
                    eng = nc.sync if k % 2 == 0 else nc.scalar
                    eng.dma_start(out=xk[k][:, csl],
                                  in_=xT[128 * k : 128 * k + 128, csl])
                if half == 0:
                    nc.scalar.dma_start(out=m01_sb, in_=m01)
            nc.scalar.dma_start(out=cc_sb[:, 512:], in_=cc[:, 512:])
            nc.scalar.dma_start(out=ss_sb[:, 512:], in_=ss[:, 512:])
            # SWDGE: q/k weights (incl. back halves, needed at k-tile 4 of
            # chunk 0) and cc/ss chunk 0 (needed at rope(0)) come before the
            # v weights (first use is ~8us later)
            wkf = wk_sb.rearrange("p k n -> p (k n)")
            wvf = wv_sb.rearrange("p k n -> p (k n)")
            nc.gpsimd.dma_start(out=wkf[:, 0:hw], in_=wkT[:, 0:hw])
            nc.gpsimd.dma_start(out=wqf[:, hw:], in_=wqT[:, hw:])
            nc.gpsimd.dma_start(out=wkf[:, hw:], in_=wkT[:, hw:])
            nc.gpsimd.dma_start(out=cc_sb[:, 0:512], in_=cc[:, 0:512])
            nc.gpsimd.dma_start(out=ss_sb[:, 0:512], in_=ss[:, 0:512])
            # bv broadcast to all partitions once; the v evacuations add it
            # on the DVE (cheaper than rank-1 bias matmuls on the PE)
            bvb_sb = const.tile([128, DSH], F32)
            nc.gpsimd.partition_broadcast(bvb_sb, bv_sb)
            nc.gpsimd.dma_start(out=wvf[:, 0:hw], in_=wvT[:, 0:hw])
            nc.gpsimd.dma_start(out=wvf[:, hw:], in_=wvT[:, hw:])

            qT_sb = persist.tile([128, 2, T], BF16)   # [d-tile, t], heads 2*dt+{0,1}
            kT_sb = persist.tile([128, 2, T], BF16)
            v1_sb = persist.tile([128, G, NTT, 65], BF16)  # [s, head, s-tile, d|1]
            # only the ones-column needs init (softmax denominators)
            nc.vector.memset(v1_sb[:, :, :, 64:65], 1.0)
            ctxT_sb = persist.tile([128, 2, T], BF16)

            # ---- emission helpers -------------------------------------------
            def qk_unit(tch, k, which):
                """One k-tile of the q or k projection of chunk tch (2 mms)."""
                tsl = slice(512 * tch, 512 * tch + 512)
                w_sb = wq_sb if which == 0 else wk_sb
                dst = qp if which == 0 else (kp23 if tch == 0 else kp01)
                xt = xk[k][:, tsl]
                for dt in range(2):
                    dsl = slice(128 * dt, 128 * dt + 128)
                    nc.tensor.matmul(dst[dt], w_sb[:, k, dsl], xt,
                                     start=(k == 0), stop=(k == KT - 1))

            def v_unit(tch, tt):
                """The full v chain for t-tile tt of chunk tch (8 mms)."""
                vt = vp2[tt % 2]
                col = 512 * tch + 128 * tt
                for k in range(KT):
                    nc.tensor.matmul(vt, xk[k][:, col : col + 128], wv_sb[:, k, :],
                                     start=(k == 0), stop=(k == KT - 1))

            def v_copy(tch, tt):
                # evacuate + add bv in fp32 (bias via a broadcast tensor_add,
                # not a rank-1 matmul: those cost ~0.3us of PE each)
                st = 4 * tch + tt
                nc.vector.tensor_add(v1_sb[:, :, st, 0:64], vp2[tt % 2], bvb_sb)

            def rope_one(which, dt, tch):
                """Bias add + RoPE for (q|k, dt) of chunk tch; frees its psum."""
                tsl = slice(512 * tch, 512 * tch + 512)
                psumt = (qp if which == 0 else (kp23 if tch == 0 else kp01))[dt]
                dst = qT_sb if which == 0 else kT_sb
                raw = ropep.tile([128, 512], BF16, tag=f"raw{which}{dt}", bufs=2)
                nc.vector.tensor_scalar_add(
                    raw, psumt, bqk_sb[:, 2 * which + dt : 2 * which + dt + 1])
                swp = ropep.tile([128, 512], BF16, tag="swp", bufs=2)
                for blk in range(4):
                    # SWDGE: off the HWDGE queues
                    sb = blk ^ 1
                    nc.gpsimd.dma_start(
                        out=swp[32 * blk : 32 * blk + 32, :],
                        in_=raw[32 * sb : 32 * sb + 32, :])
                t1 = ropep.tile([128, 512], BF16, tag="t1", bufs=2)
                t2 = ropep.tile([128, 512], BF16, tag="t2", bufs=2)
                nc.vector.tensor_mul(t1, raw, cc_sb[:, tsl])
                nc.vector.tensor_mul(t2, swp, ss_sb[:, tsl])
                nc.vector.tensor_add(dst[:, dt, tsl], t1, t2)

            # ---- the filler deque + markers ---------------------------------
            fillers = []  # (pe_cost_us, closure_or_None, marker_tag)

            def add_fill(cost, f, tag=None):
                fillers.append((cost, f, tag))

            def pop_fill(budget):
                while fillers and budget > 0:
                    cost, f, _tag = fillers.pop(0)
                    if f is not None:
                        f()
                    budget -= cost

            def force_pop_to(tag):
                if not any(t == tag for _, _, t in fillers):
                    return
                while fillers:
                    cost, f, t = fillers.pop(0)
                    if f is not None:
                        f()
                    if t == tag:
                        return

            def add_chunk_units(tch):
                """Queue chunk tch's projections (q, rope-q, k, rope-k, v)."""
                for k in range(KT):
                    add_fill(0.55, lambda k=k: qk_unit(tch, k, 0))
                for dt in range(2):
                    # charged ~a pair of budget so the kp units that reuse
                    # these banks land a couple of exp-pairs later
                    add_fill(1.2, lambda dt=dt: rope_one(0, dt, tch))
                for k in range(KT):
                    add_fill(0.55, lambda k=k: qk_unit(tch, k, 1))
                for dt in range(2):
                    add_fill(1.2, lambda dt=dt: rope_one(1, dt, tch))
                add_fill(0.0, None, f"rope{tch}")
                for tt in range(4):
                    add_fill(1.1, lambda tt=tt: v_unit(tch, tt))
                    add_fill(0.5, lambda tt=tt: v_copy(tch, tt))
                add_fill(0.0, None, f"v{tch}")

            # ---- attention position machinery (score metronome) -------------
            def emit_scores(j, dt, drain_inline=False):
                qsl = slice(512 * j, 512 * j + 512)
                nst = 4 * j + 4  # s-tiles needed (incl. diagonal)
                npairs = nst // 2
                ats = [attnp.tile([128, NTT, 512], BF16, tag=f"at{dt}{i}",
                                  name=f"at{dt}{i}", bufs=1) for i in range(2)]

                def pv_pair(hh, p2s):
                    # safe ONLY after the deque is fully drained (no pending
                    # users of banks 2,3 / ctxT / v1 left un-emitted)
                    ct, at, h = cts[hh], ats[hh], 2 * dt + hh
                    for st in (2 * p2s, 2 * p2s + 1):
                        c = max(st - 4 * j, 0)
                        nc.tensor.matmul(
                            ct[:, 128 * c :], v1_sb[:, h, st, :],
                            at[:, st, 128 * c :],
                            start=(st == 0), stop=(st == nst - 1))

                for p2 in range(npairs):  # scores + exp, 2 s-tiles a time
                    # the last pair holds diagonal s-tiles whose q-columns
                    # < 256 are fully masked: skip them
                    co = 256 if p2 == npairs - 1 else 0
                    for i in range(2):
                        st = 2 * p2 + i
                        # per-s-tile exact causal trim: q-cols < 128*(st-4j)
                        # are fully masked.  exp still covers [co:] -- the
                        # stale psum it reads there is never consumed (PV
                        # skips those columns with the same offset).
                        moff = max(co, 128 * max(st - 4 * j, 0))
                        for hh in range(2):  # rows 0-63 / 64-127
                            rsl = slice(64 * hh, 64 * hh + 64)
                            nc.tensor.matmul(
                                scs[hh][:, i, moff:],
                                kT_sb[rsl, dt, 128 * st : 128 * st + 128],
                                qT_sb[rsl, dt, 512 * j + moff : 512 * j + 512],
                                start=True, stop=True,
                                tile_position=(64 * hh, 0))
                    for hh in range(2):
                        nc.scalar.activation(
                            out=ats[hh][:, 2 * p2 : 2 * p2 + 2, co:],
                            in_=scs[hh][:, :, co:],
                            func=mybir.ActivationFunctionType.Exp, scale=0.125)
                    if drain_inline:
                        if p2 == 0:  # empty the deque under pair-0's exps
                            pop_fill(999.0)
                        elif p2 <= npairs - 2:  # non-diagonal pairs only
                            pv_pair(1, p2 - 1)
                            pv_pair(0, p2 - 1)
                    else:
                        # early positions have few pairs but a deep deque:
                        # drain more per pair so later chunks' projections
                        # spread under the exp stream, not at force-pops
                        pop_fill(3.0 if j == 0 else (2.0 if j == 1 else 1.3))
                if drain_inline:
                    # diagonal fixup, trailing (diagonal) PV pairs, epilogues
                    for hh in range(2):
                        at = ats[hh]
                        base = at[:, 4 * j, 0:128]
                        diag_ap = bass.AP(
                            tensor=base.tensor, offset=base.offset,
                            ap=[list(base.ap[0]), [640, 4], [1, 128]])
                        m01_b = bass.AP(
                            tensor=m01_sb.tensor, offset=m01_sb.offset,
                            ap=[list(m01_sb.ap[0]), [0, 4], [1, 128]])
                        nc.vector.tensor_mul(diag_ap, diag_ap, m01_b)
                    for p2s in range(npairs - 2, npairs):
                        pv_pair(1, p2s)
                        pv_pair(0, p2s)
                    emit_epi(cts[1], 1, dt, qsl)
                    emit_epi(cts[0], 0, dt, qsl)
                return ats, qsl, nst

            def make_fillers(j, dt, ats, qsl, nst):
                """PV + softmax epilogue of position (j, dt), deque units.

                Deque residency keeps the FIFO ordering of everything that
                shares PSUM banks 2,3 (ct chains, po units) and ctxT: a unit
                only ever touches state whose earlier users sit ahead of it
                in the deque.
                """
                fl = []
                for hh in (1, 0):  # hh=1 first: its ctxT write goes via a DMA
                    h = 2 * dt + hh
                    at = ats[hh]

                    def diag(at=at, j=j):
                        # causal fixup: mask the 4 diagonal blocks with one
                        # strided multiply by m01
                        base = at[:, 4 * j, 0:128]
                        diag_ap = bass.AP(
                            tensor=base.tensor, offset=base.offset,
                            ap=[list(base.ap[0]), [640, 4], [1, 128]])
                        m01_b = bass.AP(
                            tensor=m01_sb.tensor, offset=m01_sb.offset,
                            ap=[list(m01_sb.ap[0]), [0, 4], [1, 128]])
                        nc.vector.tensor_mul(diag_ap, diag_ap, m01_b)
                    fl.append((0.0, diag, None))
                    ct = cts[hh]
                    for st0 in range(0, nst, 2):
                        def pv(ct=ct, at=at, h=h, st0=st0, j=j, nst=nst):
                            for st in (st0, st0 + 1):
                                c = max(st - 4 * j, 0)
                                nc.tensor.matmul(
                                    ct[:, 128 * c :], v1_sb[:, h, st, :],
                                    at[:, st, 128 * c :],
                                    start=(st == 0), stop=(st == nst - 1))
                        fl.append((0.43, pv, None))

                    fl.append((0.05,
                               lambda ct=ct, hh=hh: emit_epi(ct, hh, dt, qsl),
                               None))
                return fl

            def emit_epi(ct, hh, dt, qsl):
                rr = epip.tile([1, 512], F32, tag="rr")
                # custom-DVE ops read SBUF only: stage the PSUM denominator
                dn = epip.tile([1, 512], F32, tag="dn")
                nc.vector.tensor_copy(dn, ct[64:65, :])
                nc.vector.reciprocal_approx_fast(out=rr, in_=dn)
                rb = epip.tile([64, 512], F32, tag="rb")
                nc.gpsimd.partition_broadcast(rb, rr)
                if hh == 0:
                    nc.vector.tensor_mul(ctxT_sb[0:64, dt, qsl], ct[0:64, :], rb)
                else:
                    stg = epip.tile([64, 512], BF16, tag="stg")
                    nc.vector.tensor_mul(stg, ct[0:64, :], rb)
                    nc.sync.dma_start(out=ctxT_sb[64:128, dt, qsl], in_=stg)

            def po_fillers(j):
                """Out-projection of chunk j (4 t-tiles x 2 n-halves)."""
                fl = []
                for u, (tt, nchk) in enumerate(
                        (tt, nchk) for tt in range(4 * j, 4 * j + 4)
                        for nchk in range(2)):
                    po = pos2[u % 2]

                    def pomm(po=po, tt=tt, nchk=nchk):
                        for k in range(2):
                            nc.tensor.matmul(
                                po,
                                ctxT_sb[:, k, 128 * tt : 128 * tt + 128],
                                wo_sb[:, k, 512 * nchk : 512 * nchk + 512],
                                start=(k == 0), stop=(k == 1))
                    fl.append((0.43, pomm, None))

                    def poev(po=po, tt=tt, nchk=nchk):
                        osb = epip.tile([128, 512], BF16, tag="osb", bufs=3)
                        nc.vector.tensor_copy(osb, po)
                        # alternate output queues so the last writes drain in
                        # parallel instead of serially on one queue
                        eng = nc.sync if (tt + nchk) % 2 == 0 else nc.scalar
                        eng.dma_start(
                            out=out[128 * tt : 128 * tt + 128,
                                    512 * nchk : 512 * nchk + 512],
                            in_=osb)
                    fl.append((0.1, poev, None))
                return fl

            # ---- prologue: warm the PE, project chunk 0 directly ------------
            wj = const.tile([128, 128], BF16)
            nc.vector.memset(wj, 0.0)
            for _ in range(24):  # junk mms release the HAM gate while x lands
                nc.tensor.matmul(scs[1][:, 1, 0:128], wj, wj, start=True, stop=True)
            for k in range(KT):
                qk_unit(0, k, 0)
            for dt in range(2):
                rope_one(0, dt, 0)
            for k in range(KT):
                qk_unit(0, k, 1)   # kp on banks 2,3: no WAR on rope-q
            for dt in range(2):
                rope_one(1, dt, 0)
            for tt in range(4):
                v_unit(0, tt)
                v_copy(0, tt)

            # ---- the fused stream -------------------------------------------
            seq = [(0, 0), (0, 1), (1, 0), (1, 1), (2, 0), (2, 1), (3, 0),
                   (3, 1)]
            last_pv = {}
            for j, dt in seq:
                if dt == 0 and j + 1 < NCH:
                    add_chunk_units(j + 1)
                if (j, dt) == (0, 0):
                    # wo arrives on the SWDGE well before po(0) pops
                    nc.gpsimd.dma_start(
                        out=wo_sb.rearrange("p k n -> p (k n)"), in_=woT)
                if j > 0:
                    # chunk j's rope must be done before its scores
                    force_pop_to(f"rope{j}")
                if dt in last_pv:
                    # the previous same-parity position's PV must have
                    # consumed the at tiles this position's exps rewrite
                    force_pop_to(last_pv[dt])
                last = (j, dt) == seq[-1]
                ats, qsl, nst = emit_scores(j, dt, drain_inline=last)
                if not last:
                    fillers += make_fillers(j, dt, ats, qsl, nst)
                    last_pv[dt] = f"pv{j}{dt}"
                    add_fill(0.0, None, last_pv[dt])
                if dt == 1:
                    fillers += po_fillers(j)
            for _cost, f, _tag in fillers:  # flush the tail
                if f is not None:
                    f()

    nc.compile()
    return nc


def _make_tables():
    i = np.arange(0, DK, 2, dtype=np.float32) / DK  # 2i/DK
    theta = 10000.0 ** i  # [32]
    pos = np.arange(T, dtype=np.float32)
    ang = pos[None, :] / theta[:, None]  # [32, T]
    sinT, cosT = np.sin(ang), np.cos(ang)
    import ml_dtypes
    cc = np.tile(cosT, (4, 1)).astype(ml_dtypes.bfloat16)  # [128, T]
    ss = np.tile(np.concatenate([-sinT, sinT], 0), (2, 1)).astype(ml_dtypes.bfloat16)
    m01 = (np.arange(128)[:, None] <= np.arange(128)[None, :]).astype(ml_dtypes.bfloat16)
    return cc, ss, m01


def _make_in_maps(x, wq, bq, wk, bk, wv, bv, wo):
    cc, ss, m01 = _make_tables()
    p = np.concatenate([np.arange(0, DK, 2), np.arange(1, DK, 2)])  # rope perm
    in_maps = []
    for core in range(NCORES):
        b, g = divmod(core, G)
        heads = np.arange(4 * g, 4 * g + 4)
        rows_qk = np.concatenate([64 * h + p for h in heads])
        rows_v = np.concatenate([64 * h + np.arange(DK) for h in heads])
        bqk = np.stack([bq[rows_qk[0:128]], bq[rows_qk[128:256]],
                        bk[rows_qk[0:128]], bk[rows_qk[128:256]]], axis=1)
        import ml_dtypes
        bf = ml_dtypes.bfloat16
        def wtile(w):  # [D, DSH] -> [128, KT*DSH] matching sbuf [p, k, n]
            return np.ascontiguousarray(
                w.reshape(KT, 128, DSH).transpose(1, 0, 2).reshape(128, KT * DSH))
        woTl = wo[:, rows_v].T.astype(bf)  # [DSH, D]
        woTl = woTl.reshape(2, 128, D).transpose(1, 0, 2).reshape(128, 2 * D)
        in_maps.append({
            "xT": np.ascontiguousarray(x[b].T.astype(bf)),
            "wqT": wtile(wq[rows_qk].T.astype(bf)),
            "wkT": wtile(wk[rows_qk].T.astype(bf)),
            "wvT": wtile(wv[rows_v].T.astype(bf)),
            "woT": np.ascontiguousarray(woTl),
            "bqk": np.ascontiguousarray(bqk.astype(np.float32)),
            "bv": np.ascontiguousarray(bv[rows_v][None, :]),
            "cc": cc, "ss": ss, "m01": m01,
            "ones": np.ones((1, 128), np.float32),
        })
    return in_maps


def _get_runner():
    """Compile once; return a jitted 8-core runner reusable across calls."""
    if "runner" in _CACHE:
        return _CACHE["runner"]
    import jax
    from jax.sharding import Mesh, PartitionSpec
    from jax.experimental.shard_map import shard_map

    install_neuronx_cc_hook()
    nc = _build_bass()

    partition_name = nc.partition_id_tensor.name if nc.partition_id_tensor else None
    in_names, out_names, out_avals = [], [], []
    for alloc in nc.m.functions[0].allocations:
        if not isinstance(alloc, mybir.MemoryLocationSet):
            continue
        name = alloc.memorylocations[0].name
        if alloc.kind == "ExternalInput":
            if name != partition_name:
                in_names.append(name)
        elif alloc.kind == "ExternalOutput":
            out_names.append(name)
            out_avals.append(
                jax.core.ShapedArray(tuple(alloc.tensor_shape), mybir.dt.np(alloc.dtype)))
    n_params = len(in_names)
    all_in = list(in_names) + list(out_names)

    def _pid():
        import jax.numpy as jnp
        from concourse.bass2jax import partition_id_tensor
        return partition_id_tensor()

    def _body(*args):
        operands = list(args)
        if partition_name is not None:
            operands.append(_pid())
        outs = _bass_exec_p.bind(
            *operands,
            out_avals=tuple(out_avals),
            in_names=tuple(all_in + ([partition_name] if partition_name else [])),
            out_names=tuple(out_names),
            lowering_input_output_aliases=(),
            sim_require_finite=True,
            sim_require_nnan=True,
            nc=nc,
        )
        return tuple(outs)

    devices = jax.devices()[:NCORES]
    mesh = Mesh(np.asarray(devices), ("core",))
    nin = n_params + len(out_names)
    sharded = jax.jit(shard_map(
        _body, mesh=mesh,
        in_specs=(PartitionSpec("core"),) * nin,
        out_specs=(PartitionSpec("core"),) * len(out_names),
        check_rep=False))

    def run(in_maps):
        concat_in = [
            np.concatenate([np.asarray(m[nm]) for m in in_maps], axis=0)
            for nm in in_names
        ]
        zeros = [np.zeros((NCORES * a.shape[0], *a.shape[1:]), a.dtype) for a in out_avals]
        out_arrs = sharded(*concat_in, *zeros)
        o = np.asarray(out_arrs[out_names.index("out")])
        return o.reshape(NCORES, T, D)

    runner = {"run": run, "sharded": sharded, "in_names": in_names,
              "out_names": out_names, "out_avals": out_avals}
    _CACHE["runner"] = runner
    return runner


def kernel(x, wq, bq, wk, bk, wv, bv, wo, bo, attn_mask):
    x = np.asarray(x, np.float32)
    in_maps = _make_in_maps(
        x, np.asarray(wq, np.float32), np.asarray(bq, np.float32),
        np.asarray(wk, np.float32), np.asarray(bk, np.float32),
        np.asarray(wv, np.float32), np.asarray(bv, np.float32),
        np.asarray(wo, np.float32))
    parts = _get_runner()["run"](in_maps)  # [8, T, D] (bf16 partials)
    parts = np.asarray(parts).astype(np.float32)
    out = parts.reshape(B, G, T, D).sum(axis=1) + np.asarray(bo, np.float32)
    return out.astype(np.float32)


# revision 34
# speedup vs baseline: 1.0471x; 1.0471x over previous
"""Multi-head attention (B=2,T=2048,D=1024,H=16,DK=64, causal, RoPE) on 8 TRN2 cores.

Sharding: data-parallel over batch (2) x tensor-parallel over heads (16 -> 4 per
core). core = 4*b + g handles batch b, heads [4g..4g+3]. RoPE tables replicated.
Host pre-transposes x and the projection weights, and permutes the q/k head dims
into [x1(32); x2(32)] blocks per head so RoPE is pure elementwise work on chip.
Each core returns a partial output projection; the host sums the 4 head-group
partials per batch and adds the output bias.

Fused schedule: the kernel is ONE softmax-paced score/exp stream (positions
(chunk j, head-pair dt) in causal order) with ALL other PE work -- later
chunks' q/k/v projections, PV chains, epilogues, out-projection -- interleaved
as cost-budgeted fillers between score pairs.  This overlaps the ACT exp
stream (~80us) with the projection phase instead of serializing the two, and
keeps the PE dense (HAM stays warm).  PSUM is hand-placed in one 8-bank tile:
  banks 0,1: qp(dt) -> kp(dt) -> v chains      (projection home, time-muxed)
  banks 2,3: ct(hh) PV accumulators / po units (attention-consumer home)
  banks 4-7: the two [128,2,512] score tiles   (metronome, double-buffered)
(chunk 0 runs before any scores exist and uses banks 2,3 for kp so its
rope never stalls the PE).
"""

import sys

for _p in ("/opt/trn_rl_repo", "/root/.axon_site/_ro/trn_rl_repo"):
    if _p not in sys.path:
        sys.path.append(_p)

import numpy as np

from concourse import bacc, tile, mybir
import concourse.bass as bass
from concourse.bass2jax import _bass_exec_p, install_neuronx_cc_hook

B, T, D, H, DK = 2, 2048, 1024, 16, 64
G = 4          # heads per core
DSH = G * DK   # 256 sharded head dims per core
NCORES = 8
KT = D // 128  # 8 contraction tiles for projections
NTT = T // 128  # 16 row tiles
NCH = T // 512  # 4 column chunks
F32 = mybir.dt.float32
F32R = mybir.dt.float32r
BF16 = mybir.dt.bfloat16

_CACHE = {}


def _build_bass():
    nc = bacc.Bacc("TRN2", target_bir_lowering=False, debug=False)

    xT = nc.dram_tensor("xT", [D, T], BF16, kind="ExternalInput").ap()
    wqT = nc.dram_tensor("wqT", [128, KT * DSH], BF16, kind="ExternalInput").ap()
    wkT = nc.dram_tensor("wkT", [128, KT * DSH], BF16, kind="ExternalInput").ap()
    wvT = nc.dram_tensor("wvT", [128, KT * DSH], BF16, kind="ExternalInput").ap()
    woT = nc.dram_tensor("woT", [128, 2 * D], BF16, kind="ExternalInput").ap()
    bqk = nc.dram_tensor("bqk", [128, 4], F32, kind="ExternalInput").ap()
    bv = nc.dram_tensor("bv", [1, DSH], F32, kind="ExternalInput").ap()
    cc = nc.dram_tensor("cc", [128, T], BF16, kind="ExternalInput").ap()
    ss = nc.dram_tensor("ss", [128, T], BF16, kind="ExternalInput").ap()
    m01 = nc.dram_tensor("m01", [128, 128], BF16, kind="ExternalInput").ap()
    ones = nc.dram_tensor("ones", [1, 128], F32, kind="ExternalInput").ap()
    out = nc.dram_tensor("out", [T, D], BF16, kind="ExternalOutput").ap()

    with tile.TileContext(nc) as tc:
        with (
            tc.tile_pool(name="const", bufs=1) as const,
            tc.tile_pool(name="persist", bufs=1) as persist,
            tc.tile_pool(name="rope", bufs=2) as ropep,
            tc.tile_pool(name="attn", bufs=2) as attnp,
            tc.tile_pool(name="epi", bufs=2) as epip,
            tc.tile_pool(name="ps", bufs=1, space="PSUM") as psp,
        ):
            # ---- the one 8-bank PSUM tile; every accumulator is a view ----
            PS = psp.tile([128, 8, 512], F32)
            qp = [PS[:, dt, :] for dt in range(2)]           # banks 0,1
            kp01 = [PS[:, dt, :] for dt in range(2)]         # banks 0,1 (chunks>=1)
            kp23 = [PS[:, 2 + dt, :] for dt in range(2)]     # banks 2,3 (chunk 0)
            vp2 = [PS[:, i, 0:256] for i in range(2)]        # banks 0,1 alternating
            cts = [PS[0:65, 2 + hh, :] for hh in range(2)]   # banks 2,3
            pos2 = [PS[:, 2 + i, :] for i in range(2)]       # banks 2,3 (po units)
            scs = [PS[:, 4 + 2 * hh : 6 + 2 * hh, :] for hh in range(2)]  # banks 4-7

            # ---- resident tensors; DMAs issued in consumption order ----
            # sync + scalar are the two HWDGE queues; gpsimd DMAs ride the
            # software DGE.  Full-row transfers (4KB/partition-line) keep the
            # descriptor count minimal -- per-dma sequencer time (~0.6us) is
            # the binding constraint, not HBM bandwidth.
            wq_sb = const.tile([128, KT, DSH], BF16)
            wk_sb = const.tile([128, KT, DSH], BF16)
            wv_sb = const.tile([128, KT, DSH], BF16)
            hw = KT // 2 * DSH
            xk = [const.tile([128, T], BF16, name=f"xk{_k}") for _k in range(KT)]
            cc_sb = const.tile([128, T], BF16)
            ss_sb = const.tile([128, T], BF16)
            bqk_sb = const.tile([128, 4], F32)
            bv_sb = const.tile([1, DSH], F32)
            m01_sb = const.tile([128, 128], BF16)
            wo_sb = const.tile([128, 2, D], BF16)
            # DMA issue order == prologue consumption order so chunk-0
            # projections run dense from ~3us behind the stream head.
            # x rows are split in column halves: the first halves (chunks
            # 0+1) arrive at 2x the row cadence.
            wqf = wq_sb.rearrange("p k n -> p (k n)")
            nc.sync.dma_start(out=wqf[:, 0:hw], in_=wqT[:, 0:hw])
            nc.sync.dma_start(out=bqk_sb, in_=bqk)
            nc.sync.dma_start(out=bv_sb, in_=bv)
            nc.scalar.dma_start(out=m01_sb, in_=m01)
            for half in range(2):
                csl = slice(1024 * half, 1024 * half + 1024)
                for k in range(KT):
                    eng = nc.sync if k % 2 == 0 else nc.scalar
                    eng.dma_start(out=xk[k][:, csl],
                                  in_=xT[128 * k : 128 * k + 128, csl])
            nc.scalar.dma_start(out=cc_sb[:, 512:], in_=cc[:, 512:])
            nc.scalar.dma_start(out=ss_sb[:, 512:], in_=ss[:, 512:])
            # SWDGE: q/k weights (incl. back halves, needed at k-tile 4 of
            # chunk 0) and cc/ss chunk 0 (needed at rope(0)) come before the
            # v weights (first use is ~8us later)
            wkf = wk_sb.rearrange("p k n -> p (k n)")
            wvf = wv_sb.rearrange("p k n -> p (k n)")
            nc.gpsimd.dma_start(out=wkf[:, 0:hw], in_=wkT[:, 0:hw])
            nc.gpsimd.dma_start(out=wqf[:, hw:], in_=wqT[:, hw:])
            nc.gpsimd.dma_start(out=wkf[:, hw:], in_=wkT[:, hw:])
            nc.gpsimd.dma_start(out=cc_sb[:, 0:512], in_=cc[:, 0:512])
            nc.gpsimd.dma_start(out=ss_sb[:, 0:512], in_=ss[:, 0:512])
            # bv broadcast to all partitions once; the v evacuations add it
            # on the DVE (cheaper than rank-1 bias matmuls on the PE)
            bvb_sb = const.tile([128, DSH], F32)
            nc.gpsimd.partition_broadcast(bvb_sb, bv_sb)
            nc.gpsimd.dma_start(out=wvf[:, 0:hw], in_=wvT[:, 0:hw])
            nc.gpsimd.dma_start(out=wvf[:, hw:], in_=wvT[:, hw:])

            qT_sb = persist.tile([128, 2, T], BF16)   # [d-tile, t], heads 2*dt+{0,1}
            kT_sb = persist.tile([128, 2, T], BF16)
            v1_sb = persist.tile([128, G, NTT, 65], BF16)  # [s, head, s-tile, d|1]
            # only the ones-column needs init (softmax denominators)
            nc.vector.memset(v1_sb[:, :, :, 64:65], 1.0)
            ctxT_sb = persist.tile([128, 2, T], BF16)

            # ---- emission helpers -------------------------------------------
            def qk_unit(tch, k, which):
                """One k-tile of the q or k projection of chunk tch (2 mms)."""
                tsl = slice(512 * tch, 512 * tch + 512)
                w_sb = wq_sb if which == 0 else wk_sb
                dst = qp if which == 0 else (kp23 if tch == 0 else kp01)
                xt = xk[k][:, tsl]
                for dt in range(2):
                    dsl = slice(128 * dt, 128 * dt + 128)
                    nc.tensor.matmul(dst[dt], w_sb[:, k, dsl], xt,
                                     start=(k == 0), stop=(k == KT - 1))

            def v_unit(tch, tt):
                """The full v chain for t-tile tt of chunk tch (8 mms)."""
                vt = vp2[tt % 2]
                col = 512 * tch + 128 * tt
                for k in range(KT):
                    nc.tensor.matmul(vt, xk[k][:, col : col + 128], wv_sb[:, k, :],
                                     start=(k == 0), stop=(k == KT - 1))

            def v_copy(tch, tt):
                # evacuate + add bv in fp32 (bias via a broadcast tensor_add,
                # not a rank-1 matmul: those cost ~0.3us of PE each)
                st = 4 * tch + tt
                nc.vector.tensor_add(v1_sb[:, :, st, 0:64], vp2[tt % 2], bvb_sb)

            def rope_one(which, dt, tch):
                """Bias add + RoPE for (q|k, dt) of chunk tch; frees its psum."""
                tsl = slice(512 * tch, 512 * tch + 512)
                psumt = (qp if which == 0 else (kp23 if tch == 0 else kp01))[dt]
                dst = qT_sb if which == 0 else kT_sb
                raw = ropep.tile([128, 512], BF16, tag=f"raw{which}{dt}", bufs=2)
                nc.vector.tensor_scalar_add(
                    raw, psumt, bqk_sb[:, 2 * which + dt : 2 * which + dt + 1])
                swp = ropep.tile([128, 512], BF16, tag="swp", bufs=2)
                for blk in range(4):
                    # SWDGE: off the HWDGE queues
                    sb = blk ^ 1
                    nc.gpsimd.dma_start(
                        out=swp[32 * blk : 32 * blk + 32, :],
                        in_=raw[32 * sb : 32 * sb + 32, :])
                t1 = ropep.tile([128, 512], BF16, tag="t1", bufs=2)
                t2 = ropep.tile([128, 512], BF16, tag="t2", bufs=2)
                nc.vector.tensor_mul(t1, raw, cc_sb[:, tsl])
                nc.vector.tensor_mul(t2, swp, ss_sb[:, tsl])
                nc.vector.tensor_add(dst[:, dt, tsl], t1, t2)

            # ---- the filler deque + markers ---------------------------------
            fillers = []  # (pe_cost_us, closure_or_None, marker_tag)

            def add_fill(cost, f, tag=None):
                fillers.append((cost, f, tag))

            def pop_fill(budget):
                while fillers and budget > 0:
                    cost, f, _tag = fillers.pop(0)
                    if f is not None:
                        f()
                    budget -= cost

            def force_pop_to(tag):
                if not any(t == tag for _, _, t in fillers):
                    return
                while fillers:
                    cost, f, t = fillers.pop(0)
                    if f is not None:
                        f()
                    if t == tag:
                        return

            def add_chunk_units(tch):
                """Queue chunk tch's projections (q, rope-q, k, rope-k, v)."""
                for k in range(KT):
                    add_fill(0.55, lambda k=k: qk_unit(tch, k, 0))
                for dt in range(2):
                    # charged ~a pair of budget so the kp units that reuse
                    # these banks land a couple of exp-pairs later
                    add_fill(1.2, lambda dt=dt: rope_one(0, dt, tch))
                for k in range(KT):
                    add_fill(0.55, lambda k=k: qk_unit(tch, k, 1))
                for dt in range(2):
                    add_fill(1.2, lambda dt=dt: rope_one(1, dt, tch))
                add_fill(0.0, None, f"rope{tch}")
                for tt in range(4):
                    add_fill(1.1, lambda tt=tt: v_unit(tch, tt))
                    add_fill(0.5, lambda tt=tt: v_copy(tch, tt))
                add_fill(0.0, None, f"v{tch}")

            # ---- attention position machinery (score metronome) -------------
            def emit_scores(j, dt):
                qsl = slice(512 * j, 512 * j + 512)
                nst = 4 * j + 4  # s-tiles needed (incl. diagonal)
                npairs = nst // 2
                ats = [attnp.tile([128, NTT, 512], BF16, tag=f"at{dt}{i}",
                                  name=f"at{dt}{i}", bufs=1) for i in range(2)]
                for p2 in range(npairs):  # scores + exp, 2 s-tiles a time
                    # the last pair holds diagonal s-tiles whose q-columns
                    # < 256 are fully masked: skip them
                    co = 256 if p2 == npairs - 1 else 0
                    for i in range(2):
                        st = 2 * p2 + i
                        # per-s-tile exact causal trim: q-cols < 128*(st-4j)
                        # are fully masked.  exp still covers [co:] -- the
                        # stale psum it reads there is never consumed (PV
                        # skips those columns with the same offset).
                        moff = max(co, 128 * max(st - 4 * j, 0))
                        for hh in range(2):  # rows 0-63 / 64-127
                            rsl = slice(64 * hh, 64 * hh + 64)
                            nc.tensor.matmul(
                                scs[hh][:, i, moff:],
                                kT_sb[rsl, dt, 128 * st : 128 * st + 128],
                                qT_sb[rsl, dt, 512 * j + moff : 512 * j + 512],
                                start=True, stop=True,
                                tile_position=(64 * hh, 0))
                    for hh in range(2):
                        nc.scalar.activation(
                            out=ats[hh][:, 2 * p2 : 2 * p2 + 2, co:],
                            in_=scs[hh][:, :, co:],
                            func=mybir.ActivationFunctionType.Exp, scale=0.125)
                    # early positions have few pairs but a deep deque: drain
                    # more per pair so later chunks' projections spread under
                    # the exp stream instead of bunching at force-pops
                    pop_fill(3.0 if j == 0 else (2.0 if j == 1 else 1.3))
                return ats, qsl, nst

            def make_fillers(j, dt, ats, qsl, nst):
                """PV + softmax epilogue of position (j, dt), deque units.

                Deque residency keeps the FIFO ordering of everything that
                shares PSUM banks 2,3 (ct chains, po units) and ctxT: a unit
                only ever touches state whose earlier users sit ahead of it
                in the deque.
                """
                fl = []
                for hh in (1, 0):  # hh=1 first: its ctxT write goes via a DMA
                    h = 2 * dt + hh
                    at = ats[hh]

                    def diag(at=at, j=j):
                        # causal fixup: mask the 4 diagonal blocks with one
                        # strided multiply by m01
                        base = at[:, 4 * j, 0:128]
                        diag_ap = bass.AP(
                            tensor=base.tensor, offset=base.offset,
                            ap=[list(base.ap[0]), [640, 4], [1, 128]])
                        m01_b = bass.AP(
                            tensor=m01_sb.tensor, offset=m01_sb.offset,
                            ap=[list(m01_sb.ap[0]), [0, 4], [1, 128]])
                        nc.vector.tensor_mul(diag_ap, diag_ap, m01_b)
                    fl.append((0.0, diag, None))
                    ct = cts[hh]
                    for st0 in range(0, nst, 2):
                        def pv(ct=ct, at=at, h=h, st0=st0, j=j, nst=nst):
                            for st in (st0, st0 + 1):
                                c = max(st - 4 * j, 0)
                                nc.tensor.matmul(
                                    ct[:, 128 * c :], v1_sb[:, h, st, :],
                                    at[:, st, 128 * c :],
                                    start=(st == 0), stop=(st == nst - 1))
                        fl.append((0.43, pv, None))

                    def epi(ct=ct, hh=hh, dt=dt, qsl=qsl):
                        rr = epip.tile([1, 512], F32, tag="rr")
                        # custom-DVE ops read SBUF only: stage the PSUM
                        # denominator row first
                        dn = epip.tile([1, 512], F32, tag="dn")
                        nc.vector.tensor_copy(dn, ct[64:65, :])
                        nc.vector.reciprocal_approx_fast(out=rr, in_=dn)
                        rb = epip.tile([64, 512], F32, tag="rb")
                        nc.gpsimd.partition_broadcast(rb, rr)
                        if hh == 0:
                            nc.vector.tensor_mul(ctxT_sb[0:64, dt, qsl], ct[0:64, :], rb)
                        else:
                            stg = epip.tile([64, 512], BF16, tag="stg")
                            nc.vector.tensor_mul(stg, ct[0:64, :], rb)
                            nc.sync.dma_start(out=ctxT_sb[64:128, dt, qsl], in_=stg)
                    fl.append((0.05, epi, None))
                return fl

            def po_fillers(j):
                """Out-projection of chunk j (4 t-tiles x 2 n-halves)."""
                fl = []
                for u, (tt, nchk) in enumerate(
                        (tt, nchk) for tt in range(4 * j, 4 * j + 4)
                        for nchk in range(2)):
                    po = pos2[u % 2]

                    def pomm(po=po, tt=tt, nchk=nchk):
                        for k in range(2):
                            nc.tensor.matmul(
                                po,
                                ctxT_sb[:, k, 128 * tt : 128 * tt + 128],
                                wo_sb[:, k, 512 * nchk : 512 * nchk + 512],
                                start=(k == 0), stop=(k == 1))
                    fl.append((0.43, pomm, None))

                    def poev(po=po, tt=tt, nchk=nchk):
                        osb = epip.tile([128, 512], BF16, tag="osb", bufs=3)
                        nc.vector.tensor_copy(osb, po)
                        # alternate output queues so the last writes drain in
                        # parallel instead of serially on one queue
                        eng = nc.sync if (tt + nchk) % 2 == 0 else nc.scalar
                        eng.dma_start(
                            out=out[128 * tt : 128 * tt + 128,
                                    512 * nchk : 512 * nchk + 512],
                            in_=osb)
                    fl.append((0.1, poev, None))
                return fl

            # ---- prologue: warm the PE, project chunk 0 directly ------------
            wj = const.tile([128, 128], BF16)
            nc.vector.memset(wj, 0.0)
            for _ in range(24):  # junk mms release the HAM gate while x lands
                nc.tensor.matmul(scs[1][:, 1, 0:128], wj, wj, start=True, stop=True)
            for k in range(KT):
                qk_unit(0, k, 0)
            for dt in range(2):
                rope_one(0, dt, 0)
            for k in range(KT):
                qk_unit(0, k, 1)   # kp on banks 2,3: no WAR on rope-q
            for dt in range(2):
                rope_one(1, dt, 0)
            for tt in range(4):
                v_unit(0, tt)
                v_copy(0, tt)

            # ---- the fused stream -------------------------------------------
            for j in range(NCH):
                for dt in range(2):
                    if dt == 0 and j + 1 < NCH:
                        add_chunk_units(j + 1)
                    if dt == 0 and j == 0:
                        # wo arrives on the SWDGE well before po(0) pops
                        nc.gpsimd.dma_start(
                            out=wo_sb.rearrange("p k n -> p (k n)"), in_=woT)
                    if j > 0:
                        # chunk j's rope must be done before its scores, and
                        # the previous same-parity position's PV must have
                        # consumed the at tiles this position's exps rewrite
                        force_pop_to(f"rope{j}")
                        force_pop_to(f"pv{j-1}{dt}")
                    ats, qsl, nst = emit_scores(j, dt)
                    fillers += make_fillers(j, dt, ats, qsl, nst)
                    add_fill(0.0, None, f"pv{j}{dt}")
                    if dt == 1:
                        fillers += po_fillers(j)
            for _cost, f, _tag in fillers:  # flush the tail
                if f is not None:
                    f()

    nc.compile()
    return nc


def _make_tables():
    i = np.arange(0, DK, 2, dtype=np.float32) / DK  # 2i/DK
    theta = 10000.0 ** i  # [32]
    pos = np.arange(T, dtype=np.float32)
    ang = pos[None, :] / theta[:, None]  # [32, T]
    sinT, cosT = np.sin(ang), np.cos(ang)
    import ml_dtypes
    cc = np.tile(cosT, (4, 1)).astype(ml_dtypes.bfloat16)  # [128, T]
    ss = np.tile(np.concatenate([-sinT, sinT], 0), (2, 1)).astype(ml_dtypes.bfloat16)
    m01 = (np.arange(128)[:, None] <= np.arange(128)[None, :]).astype(ml_dtypes.bfloat16)
    return cc, ss, m01


def _make_in_maps(x, wq, bq, wk, bk, wv, bv, wo):
    cc, ss, m01 = _make_tables()
    p = np.concatenate([np.arange(0, DK, 2), np.arange(1, DK, 2)])  # rope perm
    in_maps = []
    for core in range(NCORES):
        b, g = divmod(core, G)
        heads = np.arange(4 * g, 4 * g + 4)
        rows_qk = np.concatenate([64 * h + p for h in heads])
        rows_v = np.concatenate([64 * h + np.arange(DK) for h in heads])
        bqk = np.stack([bq[rows_qk[0:128]], bq[rows_qk[128:256]],
                        bk[rows_qk[0:128]], bk[rows_qk[128:256]]], axis=1)
        import ml_dtypes
        bf = ml_dtypes.bfloat16
        def wtile(w):  # [D, DSH] -> [128, KT*DSH] matching sbuf [p, k, n]
            return np.ascontiguousarray(
                w.reshape(KT, 128, DSH).transpose(1, 0, 2).reshape(128, KT * DSH))
        woTl = wo[:, rows_v].T.astype(bf)  # [DSH, D]
        woTl = woTl.reshape(2, 128, D).transpose(1, 0, 2).reshape(128, 2 * D)
        in_maps.append({
            "xT": np.ascontiguousarray(x[b].T.astype(bf)),
            "wqT": wtile(wq[rows_qk].T.astype(bf)),
            "wkT": wtile(wk[rows_qk].T.astype(bf)),
            "wvT": wtile(wv[rows_v].T.astype(bf)),
            "woT": np.ascontiguousarray(woTl),
            "bqk": np.ascontiguousarray(bqk.astype(np.float32)),
            "bv": np.ascontiguousarray(bv[rows_v][None, :]),
            "cc": cc, "ss": ss, "m01": m01,
            "ones": np.ones((1, 128), np.float32),
        })
    return in_maps


def _get_runner():
    """Compile once; return a jitted 8-core runner reusable across calls."""
    if "runner" in _CACHE:
        return _CACHE["runner"]
    import jax
    from jax.sharding import Mesh, PartitionSpec
    from jax.experimental.shard_map import shard_map

    install_neuronx_cc_hook()
    nc = _build_bass()

    partition_name = nc.partition_id_tensor.name if nc.partition_id_tensor else None
    in_names, out_names, out_avals = [], [], []
    for alloc in nc.m.functions[0].allocations:
        if not isinstance(alloc, mybir.MemoryLocationSet):
            continue
        name = alloc.memorylocations[0].name
        if alloc.kind == "ExternalInput":
            if name != partition_name:
                in_names.append(name)
        elif alloc.kind == "ExternalOutput":
            out_names.append(name)
            out_avals.append(
                jax.core.ShapedArray(tuple(alloc.tensor_shape), mybir.dt.np(alloc.dtype)))
    n_params = len(in_names)
    all_in = list(in_names) + list(out_names)

    def _pid():
        import jax.numpy as jnp
        from concourse.bass2jax import partition_id_tensor
        return partition_id_tensor()

    def _body(*args):
        operands = list(args)
        if partition_name is not None:
            operands.append(_pid())
        outs = _bass_exec_p.bind(
            *operands,
            out_avals=tuple(out_avals),
            in_names=tuple(all_in + ([partition_name] if partition_name else [])),
            out_names=tuple(out_names),
            lowering_input_output_aliases=(),
            sim_require_finite=True,
            sim_require_nnan=True,
            nc=nc,
        )
        return tuple(outs)

    devices = jax.devices()[:NCORES]
    mesh = Mesh(np.asarray(devices), ("core",))
    nin = n_params + len(out_names)
    sharded = jax.jit(shard_map(
        _body, mesh=mesh,
        in_specs=(PartitionSpec("core"),) * nin,
        out_specs=(PartitionSpec("core"),) * len(out_names),
        check_rep=False))

    def run(in_maps):
        concat_in = [
            np.concatenate([np.asarray(m[nm]) for m in in_maps], axis=0)
            for nm in in_names
        ]
        zeros = [np.zeros((NCORES * a.shape[0], *a.shape[1:]), a.dtype) for a in out_avals]
        out_arrs = sharded(*concat_in, *zeros)
        o = np.asarray(out_arrs[out_names.index("out")])
        return o.reshape(NCORES, T, D)

    runner = {"run": run, "sharded": sharded, "in_names": in_names,
              "out_names": out_names, "out_avals": out_avals}
    _CACHE["runner"] = runner
    return runner


def kernel(x, wq, bq, wk, bk, wv, bv, wo, bo, attn_mask):
    x = np.asarray(x, np.float32)
    in_maps = _make_in_maps(
        x, np.asarray(wq, np.float32), np.asarray(bq, np.float32),
        np.asarray(wk, np.float32), np.asarray(bk, np.float32),
        np.asarray(wv, np.float32), np.asarray(bv, np.float32),
        np.asarray(wo, np.float32))
    parts = _get_runner()["run"](in_maps)  # [8, T, D] (bf16 partials)
    parts = np.asarray(parts).astype(np.float32)
    out = parts.reshape(B, G, T, D).sum(axis=1) + np.asarray(bo, np.float32)
    return out.astype(np.float32)


# revision 35
# speedup vs baseline: 1.0572x; 1.0096x over previous
"""Multi-head attention (B=2,T=2048,D=1024,H=16,DK=64, causal, RoPE) on 8 TRN2 cores.

Sharding: data-parallel over batch (2) x tensor-parallel over heads (16 -> 4 per
core). core = 4*b + g handles batch b, heads [4g..4g+3]. RoPE tables replicated.
Host pre-transposes x and the projection weights, and permutes the q/k head dims
into [x1(32); x2(32)] blocks per head so RoPE is pure elementwise work on chip.
Each core returns a partial output projection; the host sums the 4 head-group
partials per batch and adds the output bias.

Fused schedule: the kernel is ONE softmax-paced score/exp stream (positions
(chunk j, head-pair dt) in causal order) with ALL other PE work -- later
chunks' q/k/v projections, PV chains, epilogues, out-projection -- interleaved
as cost-budgeted fillers between score pairs.  This overlaps the ACT exp
stream (~80us) with the projection phase instead of serializing the two, and
keeps the PE dense (HAM stays warm).  PSUM is hand-placed in one 8-bank tile:
  banks 0,1: qp(dt) -> kp(dt) -> v chains      (projection home, time-muxed)
  banks 2,3: ct(hh) PV accumulators / po units (attention-consumer home)
  banks 4-7: the two [128,2,512] score tiles   (metronome, double-buffered)
(chunk 0 runs before any scores exist and uses banks 2,3 for kp so its
rope never stalls the PE).
"""

import sys

for _p in ("/opt/trn_rl_repo", "/root/.axon_site/_ro/trn_rl_repo"):
    if _p not in sys.path:
        sys.path.append(_p)

import numpy as np

from concourse import bacc, tile, mybir
import concourse.bass as bass
from concourse.bass2jax import _bass_exec_p, install_neuronx_cc_hook

B, T, D, H, DK = 2, 2048, 1024, 16, 64
G = 4          # heads per core
DSH = G * DK   # 256 sharded head dims per core
NCORES = 8
KT = D // 128  # 8 contraction tiles for projections
NTT = T // 128  # 16 row tiles
NCH = T // 512  # 4 column chunks
F32 = mybir.dt.float32
F32R = mybir.dt.float32r
BF16 = mybir.dt.bfloat16

_CACHE = {}


def _build_bass():
    nc = bacc.Bacc("TRN2", target_bir_lowering=False, debug=False)

    xT = nc.dram_tensor("xT", [D, T], BF16, kind="ExternalInput").ap()
    wqT = nc.dram_tensor("wqT", [128, KT * DSH], BF16, kind="ExternalInput").ap()
    wkT = nc.dram_tensor("wkT", [128, KT * DSH], BF16, kind="ExternalInput").ap()
    wvT = nc.dram_tensor("wvT", [128, KT * DSH], BF16, kind="ExternalInput").ap()
    woT = nc.dram_tensor("woT", [128, 2 * D], BF16, kind="ExternalInput").ap()
    bqk = nc.dram_tensor("bqk", [128, 4], F32, kind="ExternalInput").ap()
    bv = nc.dram_tensor("bv", [1, DSH], F32, kind="ExternalInput").ap()
    cc = nc.dram_tensor("cc", [128, T], BF16, kind="ExternalInput").ap()
    ss = nc.dram_tensor("ss", [128, T], BF16, kind="ExternalInput").ap()
    m01 = nc.dram_tensor("m01", [128, 128], BF16, kind="ExternalInput").ap()
    ones = nc.dram_tensor("ones", [1, 128], F32, kind="ExternalInput").ap()
    out = nc.dram_tensor("out", [T, D], BF16, kind="ExternalOutput").ap()

    with tile.TileContext(nc) as tc:
        with (
            tc.tile_pool(name="const", bufs=1) as const,
            tc.tile_pool(name="persist", bufs=1) as persist,
            tc.tile_pool(name="rope", bufs=2) as ropep,
            tc.tile_pool(name="attn", bufs=2) as attnp,
            tc.tile_pool(name="epi", bufs=2) as epip,
            tc.tile_pool(name="ps", bufs=1, space="PSUM") as psp,
        ):
            # ---- the one 8-bank PSUM tile; every accumulator is a view ----
            PS = psp.tile([128, 8, 512], F32)
            qp = [PS[:, dt, :] for dt in range(2)]           # banks 0,1
            kp01 = [PS[:, dt, :] for dt in range(2)]         # banks 0,1 (chunks>=1)
            kp23 = [PS[:, 2 + dt, :] for dt in range(2)]     # banks 2,3 (chunk 0)
            vp2 = [PS[:, i, 0:256] for i in range(2)]        # banks 0,1 alternating
            cts = [PS[0:65, 2 + hh, :] for hh in range(2)]   # banks 2,3
            pos2 = [PS[:, 2 + i, :] for i in range(2)]       # banks 2,3 (po units)
            scs = [PS[:, 4 + 2 * hh : 6 + 2 * hh, :] for hh in range(2)]  # banks 4-7

            # ---- resident tensors; DMAs issued in consumption order ----
            # sync + scalar are the two HWDGE queues; gpsimd DMAs ride the
            # software DGE.  Full-row transfers (4KB/partition-line) keep the
            # descriptor count minimal -- per-dma sequencer time (~0.6us) is
            # the binding constraint, not HBM bandwidth.
            wq_sb = const.tile([128, KT, DSH], BF16)
            wk_sb = const.tile([128, KT, DSH], BF16)
            wv_sb = const.tile([128, KT, DSH], BF16)
            hw = KT // 2 * DSH
            xk = [const.tile([128, T], BF16, name=f"xk{_k}") for _k in range(KT)]
            cc_sb = const.tile([128, T], BF16)
            ss_sb = const.tile([128, T], BF16)
            bqk_sb = const.tile([128, 4], F32)
            bv_sb = const.tile([1, DSH], F32)
            m01_sb = const.tile([128, 128], BF16)
            wo_sb = const.tile([128, 2, D], BF16)
            # DMA issue order == prologue consumption order so chunk-0
            # projections run dense from ~3us behind the stream head.
            # x rows are split in column halves: the first halves (chunks
            # 0+1) arrive at 2x the row cadence.
            wqf = wq_sb.rearrange("p k n -> p (k n)")
            nc.sync.dma_start(out=wqf[:, 0:hw], in_=wqT[:, 0:hw])
            nc.sync.dma_start(out=bqk_sb, in_=bqk)
            nc.sync.dma_start(out=bv_sb, in_=bv)
            nc.scalar.dma_start(out=m01_sb, in_=m01)
            for half in range(2):
                csl = slice(1024 * half, 1024 * half + 1024)
                for k in range(KT):
                    eng = nc.sync if k % 2 == 0 else nc.scalar
                    eng.dma_start(out=xk[k][:, csl],
                                  in_=xT[128 * k : 128 * k + 128, csl])
            nc.scalar.dma_start(out=cc_sb[:, 512:], in_=cc[:, 512:])
            nc.scalar.dma_start(out=ss_sb[:, 512:], in_=ss[:, 512:])
            # SWDGE: q/k weights (incl. back halves, needed at k-tile 4 of
            # chunk 0) and cc/ss chunk 0 (needed at rope(0)) come before the
            # v weights (first use is ~8us later)
            wkf = wk_sb.rearrange("p k n -> p (k n)")
            wvf = wv_sb.rearrange("p k n -> p (k n)")
            nc.gpsimd.dma_start(out=wkf[:, 0:hw], in_=wkT[:, 0:hw])
            nc.gpsimd.dma_start(out=wqf[:, hw:], in_=wqT[:, hw:])
            nc.gpsimd.dma_start(out=wkf[:, hw:], in_=wkT[:, hw:])
            nc.gpsimd.dma_start(out=cc_sb[:, 0:512], in_=cc[:, 0:512])
            nc.gpsimd.dma_start(out=ss_sb[:, 0:512], in_=ss[:, 0:512])
            # bv broadcast to all partitions once; the v evacuations add it
            # on the DVE (cheaper than rank-1 bias matmuls on the PE)
            bvb_sb = const.tile([128, DSH], F32)
            nc.gpsimd.partition_broadcast(bvb_sb, bv_sb)
            nc.gpsimd.dma_start(out=wvf[:, 0:hw], in_=wvT[:, 0:hw])
            nc.gpsimd.dma_start(out=wvf[:, hw:], in_=wvT[:, hw:])

            qT_sb = persist.tile([128, 2, T], BF16)   # [d-tile, t], heads 2*dt+{0,1}
            kT_sb = persist.tile([128, 2, T], BF16)
            v1_sb = persist.tile([128, G, NTT, 65], BF16)  # [s, head, s-tile, d|1]
            # only the ones-column needs init (softmax denominators)
            nc.vector.memset(v1_sb[:, :, :, 64:65], 1.0)
            ctxT_sb = persist.tile([128, 2, T], BF16)

            # ---- emission helpers -------------------------------------------
            def qk_unit(tch, k, which):
                """One k-tile of the q or k projection of chunk tch (2 mms)."""
                tsl = slice(512 * tch, 512 * tch + 512)
                w_sb = wq_sb if which == 0 else wk_sb
                dst = qp if which == 0 else (kp23 if tch == 0 else kp01)
                xt = xk[k][:, tsl]
                for dt in range(2):
                    dsl = slice(128 * dt, 128 * dt + 128)
                    nc.tensor.matmul(dst[dt], w_sb[:, k, dsl], xt,
                                     start=(k == 0), stop=(k == KT - 1))

            def v_unit(tch, tt):
                """The full v chain for t-tile tt of chunk tch (8 mms)."""
                vt = vp2[tt % 2]
                col = 512 * tch + 128 * tt
                for k in range(KT):
                    nc.tensor.matmul(vt, xk[k][:, col : col + 128], wv_sb[:, k, :],
                                     start=(k == 0), stop=(k == KT - 1))

            def v_copy(tch, tt):
                # evacuate + add bv in fp32 (bias via a broadcast tensor_add,
                # not a rank-1 matmul: those cost ~0.3us of PE each)
                st = 4 * tch + tt
                nc.vector.tensor_add(v1_sb[:, :, st, 0:64], vp2[tt % 2], bvb_sb)

            def rope_one(which, dt, tch):
                """Bias add + RoPE for (q|k, dt) of chunk tch; frees its psum."""
                tsl = slice(512 * tch, 512 * tch + 512)
                psumt = (qp if which == 0 else (kp23 if tch == 0 else kp01))[dt]
                dst = qT_sb if which == 0 else kT_sb
                raw = ropep.tile([128, 512], BF16, tag=f"raw{which}{dt}", bufs=2)
                nc.vector.tensor_scalar_add(
                    raw, psumt, bqk_sb[:, 2 * which + dt : 2 * which + dt + 1])
                swp = ropep.tile([128, 512], BF16, tag="swp", bufs=2)
                for blk in range(4):
                    # SWDGE: off the HWDGE queues
                    sb = blk ^ 1
                    nc.gpsimd.dma_start(
                        out=swp[32 * blk : 32 * blk + 32, :],
                        in_=raw[32 * sb : 32 * sb + 32, :])
                t1 = ropep.tile([128, 512], BF16, tag="t1", bufs=2)
                t2 = ropep.tile([128, 512], BF16, tag="t2", bufs=2)
                nc.vector.tensor_mul(t1, raw, cc_sb[:, tsl])
                nc.vector.tensor_mul(t2, swp, ss_sb[:, tsl])
                nc.vector.tensor_add(dst[:, dt, tsl], t1, t2)

            # ---- the filler deque + markers ---------------------------------
            fillers = []  # (pe_cost_us, closure_or_None, marker_tag)

            def add_fill(cost, f, tag=None):
                fillers.append((cost, f, tag))

            def pop_fill(budget):
                while fillers and budget > 0:
                    cost, f, _tag = fillers.pop(0)
                    if f is not None:
                        f()
                    budget -= cost

            def force_pop_to(tag):
                if not any(t == tag for _, _, t in fillers):
                    return
                while fillers:
                    cost, f, t = fillers.pop(0)
                    if f is not None:
                        f()
                    if t == tag:
                        return

            def add_chunk_units(tch):
                """Queue chunk tch's projections (q, rope-q, k, rope-k, v)."""
                for k in range(KT):
                    add_fill(0.55, lambda k=k: qk_unit(tch, k, 0))
                for dt in range(2):
                    # charged ~a pair of budget so the kp units that reuse
                    # these banks land a couple of exp-pairs later
                    add_fill(1.2, lambda dt=dt: rope_one(0, dt, tch))
                for k in range(KT):
                    add_fill(0.55, lambda k=k: qk_unit(tch, k, 1))
                for dt in range(2):
                    add_fill(1.2, lambda dt=dt: rope_one(1, dt, tch))
                add_fill(0.0, None, f"rope{tch}")
                for tt in range(4):
                    add_fill(1.1, lambda tt=tt: v_unit(tch, tt))
                    add_fill(0.5, lambda tt=tt: v_copy(tch, tt))
                add_fill(0.0, None, f"v{tch}")

            # ---- attention position machinery (score metronome) -------------
            def emit_scores(j, dt):
                qsl = slice(512 * j, 512 * j + 512)
                nst = 4 * j + 4  # s-tiles needed (incl. diagonal)
                npairs = nst // 2
                ats = [attnp.tile([128, NTT, 512], BF16, tag=f"at{dt}{i}",
                                  name=f"at{dt}{i}", bufs=1) for i in range(2)]
                for p2 in range(npairs):  # scores + exp, 2 s-tiles a time
                    # the last pair holds diagonal s-tiles whose q-columns
                    # < 256 are fully masked: skip them
                    co = 256 if p2 == npairs - 1 else 0
                    for i in range(2):
                        st = 2 * p2 + i
                        # per-s-tile exact causal trim: q-cols < 128*(st-4j)
                        # are fully masked.  exp still covers [co:] -- the
                        # stale psum it reads there is never consumed (PV
                        # skips those columns with the same offset).
                        moff = max(co, 128 * max(st - 4 * j, 0))
                        for hh in range(2):  # rows 0-63 / 64-127
                            rsl = slice(64 * hh, 64 * hh + 64)
                            nc.tensor.matmul(
                                scs[hh][:, i, moff:],
                                kT_sb[rsl, dt, 128 * st : 128 * st + 128],
                                qT_sb[rsl, dt, 512 * j + moff : 512 * j + 512],
                                start=True, stop=True,
                                tile_position=(64 * hh, 0))
                    for hh in range(2):
                        nc.scalar.activation(
                            out=ats[hh][:, 2 * p2 : 2 * p2 + 2, co:],
                            in_=scs[hh][:, :, co:],
                            func=mybir.ActivationFunctionType.Exp, scale=0.125)
                    # early positions have few pairs but a deep deque: drain
                    # more per pair so later chunks' projections spread under
                    # the exp stream instead of bunching at force-pops
                    pop_fill(3.0 if j == 0 else (2.0 if j == 1 else 1.5))
                return ats, qsl, nst

            def make_fillers(j, dt, ats, qsl, nst):
                """PV + softmax epilogue of position (j, dt), deque units.

                Deque residency keeps the FIFO ordering of everything that
                shares PSUM banks 2,3 (ct chains, po units) and ctxT: a unit
                only ever touches state whose earlier users sit ahead of it
                in the deque.
                """
                fl = []
                for hh in (1, 0):  # hh=1 first: its ctxT write goes via a DMA
                    h = 2 * dt + hh
                    at = ats[hh]

                    def diag(at=at, j=j):
                        # causal fixup: mask the 4 diagonal blocks with one
                        # strided multiply by m01
                        base = at[:, 4 * j, 0:128]
                        diag_ap = bass.AP(
                            tensor=base.tensor, offset=base.offset,
                            ap=[list(base.ap[0]), [640, 4], [1, 128]])
                        m01_b = bass.AP(
                            tensor=m01_sb.tensor, offset=m01_sb.offset,
                            ap=[list(m01_sb.ap[0]), [0, 4], [1, 128]])
                        nc.vector.tensor_mul(diag_ap, diag_ap, m01_b)
                    fl.append((0.0, diag, None))
                    ct = cts[hh]
                    for st0 in range(0, nst, 2):
                        def pv(ct=ct, at=at, h=h, st0=st0, j=j, nst=nst):
                            for st in (st0, st0 + 1):
                                c = max(st - 4 * j, 0)
                                nc.tensor.matmul(
                                    ct[:, 128 * c :], v1_sb[:, h, st, :],
                                    at[:, st, 128 * c :],
                                    start=(st == 0), stop=(st == nst - 1))
                        fl.append((0.43, pv, None))

                    def epi(ct=ct, hh=hh, dt=dt, qsl=qsl):
                        rr = epip.tile([1, 512], F32, tag="rr")
                        # custom-DVE ops read SBUF only: stage the PSUM
                        # denominator row first
                        dn = epip.tile([1, 512], F32, tag="dn")
                        nc.vector.tensor_copy(dn, ct[64:65, :])
                        nc.vector.reciprocal_approx_fast(out=rr, in_=dn)
                        rb = epip.tile([64, 512], F32, tag="rb")
                        nc.gpsimd.partition_broadcast(rb, rr)
                        if hh == 0:
                            nc.vector.tensor_mul(ctxT_sb[0:64, dt, qsl], ct[0:64, :], rb)
                        else:
                            stg = epip.tile([64, 512], BF16, tag="stg")
                            nc.vector.tensor_mul(stg, ct[0:64, :], rb)
                            nc.sync.dma_start(out=ctxT_sb[64:128, dt, qsl], in_=stg)
                    fl.append((0.05, epi, None))
                return fl

            def po_fillers(j):
                """Out-projection of chunk j (4 t-tiles x 2 n-halves)."""
                fl = []
                for u, (tt, nchk) in enumerate(
                        (tt, nchk) for tt in range(4 * j, 4 * j + 4)
                        for nchk in range(2)):
                    po = pos2[u % 2]

                    def pomm(po=po, tt=tt, nchk=nchk):
                        for k in range(2):
                            nc.tensor.matmul(
                                po,
                                ctxT_sb[:, k, 128 * tt : 128 * tt + 128],
                                wo_sb[:, k, 512 * nchk : 512 * nchk + 512],
                                start=(k == 0), stop=(k == 1))
                    fl.append((0.43, pomm, None))

                    def poev(po=po, tt=tt, nchk=nchk):
                        osb = epip.tile([128, 512], BF16, tag="osb", bufs=3)
                        nc.vector.tensor_copy(osb, po)
                        # alternate output queues so the last writes drain in
                        # parallel instead of serially on one queue
                        eng = nc.sync if (tt + nchk) % 2 == 0 else nc.scalar
                        eng.dma_start(
                            out=out[128 * tt : 128 * tt + 128,
                                    512 * nchk : 512 * nchk + 512],
                            in_=osb)
                    fl.append((0.1, poev, None))
                return fl

            # ---- prologue: warm the PE, project chunk 0 directly ------------
            wj = const.tile([128, 128], BF16)
            nc.vector.memset(wj, 0.0)
            for _ in range(24):  # junk mms release the HAM gate while x lands
                nc.tensor.matmul(scs[1][:, 1, 0:128], wj, wj, start=True, stop=True)
            for k in range(KT):
                qk_unit(0, k, 0)
            for dt in range(2):
                rope_one(0, dt, 0)
            for k in range(KT):
                qk_unit(0, k, 1)   # kp on banks 2,3: no WAR on rope-q
            for dt in range(2):
                rope_one(1, dt, 0)
            for tt in range(4):
                v_unit(0, tt)
                v_copy(0, tt)

            # ---- the fused stream -------------------------------------------
            for j in range(NCH):
                for dt in range(2):
                    if dt == 0 and j + 1 < NCH:
                        add_chunk_units(j + 1)
                    if dt == 0 and j == 0:
                        # wo arrives on the SWDGE well before po(0) pops
                        nc.gpsimd.dma_start(
                            out=wo_sb.rearrange("p k n -> p (k n)"), in_=woT)
                    if j > 0:
                        # chunk j's rope must be done before its scores, and
                        # the previous same-parity position's PV must have
                        # consumed the at tiles this position's exps rewrite
                        force_pop_to(f"rope{j}")
                        force_pop_to(f"pv{j-1}{dt}")
                    ats, qsl, nst = emit_scores(j, dt)
                    fillers += make_fillers(j, dt, ats, qsl, nst)
                    add_fill(0.0, None, f"pv{j}{dt}")
                    if dt == 1:
                        fillers += po_fillers(j)
            for _cost, f, _tag in fillers:  # flush the tail
                if f is not None:
                    f()

    nc.compile()
    return nc


def _make_tables():
    i = np.arange(0, DK, 2, dtype=np.float32) / DK  # 2i/DK
    theta = 10000.0 ** i  # [32]
    pos = np.arange(T, dtype=np.float32)
    ang = pos[None, :] / theta[:, None]  # [32, T]
    sinT, cosT = np.sin(ang), np.cos(ang)
    import ml_dtypes
    cc = np.tile(cosT, (4, 1)).astype(ml_dtypes.bfloat16)  # [128, T]
    ss = np.tile(np.concatenate([-sinT, sinT], 0), (2, 1)).astype(ml_dtypes.bfloat16)
    m01 = (np.arange(128)[:, None] <= np.arange(128)[None, :]).astype(ml_dtypes.bfloat16)
    return cc, ss, m01


def _make_in_maps(x, wq, bq, wk, bk, wv, bv, wo):
    cc, ss, m01 = _make_tables()
    p = np.concatenate([np.arange(0, DK, 2), np.arange(1, DK, 2)])  # rope perm
    in_maps = []
    for core in range(NCORES):
        b, g = divmod(core, G)
        heads = np.arange(4 * g, 4 * g + 4)
        rows_qk = np.concatenate([64 * h + p for h in heads])
        rows_v = np.concatenate([64 * h + np.arange(DK) for h in heads])
        bqk = np.stack([bq[rows_qk[0:128]], bq[rows_qk[128:256]],
                        bk[rows_qk[0:128]], bk[rows_qk[128:256]]], axis=1)
        import ml_dtypes
        bf = ml_dtypes.bfloat16
        def wtile(w):  # [D, DSH] -> [128, KT*DSH] matching sbuf [p, k, n]
            return np.ascontiguousarray(
                w.reshape(KT, 128, DSH).transpose(1, 0, 2).reshape(128, KT * DSH))
        woTl = wo[:, rows_v].T.astype(bf)  # [DSH, D]
        woTl = woTl.reshape(2, 128, D).transpose(1, 0, 2).reshape(128, 2 * D)
        in_maps.append({
            "xT": np.ascontiguousarray(x[b].T.astype(bf)),
            "wqT": wtile(wq[rows_qk].T.astype(bf)),
            "wkT": wtile(wk[rows_qk].T.astype(bf)),
            "wvT": wtile(wv[rows_v].T.astype(bf)),
            "woT": np.ascontiguousarray(woTl),
            "bqk": np.ascontiguousarray(bqk.astype(np.float32)),
            "bv": np.ascontiguousarray(bv[rows_v][None, :]),
            "cc": cc, "ss": ss, "m01": m01,
            "ones": np.ones((1, 128), np.float32),
        })
    return in_maps


def _get_runner():
    """Compile once; return a jitted 8-core runner reusable across calls."""
    if "runner" in _CACHE:
        return _CACHE["runner"]
    import jax
    from jax.sharding import Mesh, PartitionSpec
    from jax.experimental.shard_map import shard_map

    install_neuronx_cc_hook()
    nc = _build_bass()

    partition_name = nc.partition_id_tensor.name if nc.partition_id_tensor else None
    in_names, out_names, out_avals = [], [], []
    for alloc in nc.m.functions[0].allocations:
        if not isinstance(alloc, mybir.MemoryLocationSet):
            continue
        name = alloc.memorylocations[0].name
        if alloc.kind == "ExternalInput":
            if name != partition_name:
                in_names.append(name)
        elif alloc.kind == "ExternalOutput":
            out_names.append(name)
            out_avals.append(
                jax.core.ShapedArray(tuple(alloc.tensor_shape), mybir.dt.np(alloc.dtype)))
    n_params = len(in_names)
    all_in = list(in_names) + list(out_names)

    def _pid():
        import jax.numpy as jnp
        from concourse.bass2jax import partition_id_tensor
        return partition_id_tensor()

    def _body(*args):
        operands = list(args)
        if partition_name is not None:
            operands.append(_pid())
        outs = _bass_exec_p.bind(
            *operands,
            out_avals=tuple(out_avals),
            in_names=tuple(all_in + ([partition_name] if partition_name else [])),
            out_names=tuple(out_names),
            lowering_input_output_aliases=(),
            sim_require_finite=True,
            sim_require_nnan=True,
            nc=nc,
        )
        return tuple(outs)

    devices = jax.devices()[:NCORES]
    mesh = Mesh(np.asarray(devices), ("core",))
    nin = n_params + len(out_names)
    sharded = jax.jit(shard_map(
        _body, mesh=mesh,
        in_specs=(PartitionSpec("core"),) * nin,
        out_specs=(PartitionSpec("core"),) * len(out_names),
        check_rep=False))

    def run(in_maps):
        concat_in = [
            np.concatenate([np.asarray(m[nm]) for m in in_maps], axis=0)
            for nm in in_names
        ]
        zeros = [np.zeros((NCORES * a.shape[0], *a.shape[1:]), a.dtype) for a in out_avals]
        out_arrs = sharded(*concat_in, *zeros)
        o = np.asarray(out_arrs[out_names.index("out")])
        return o.reshape(NCORES, T, D)

    runner = {"run": run, "sharded": sharded, "in_names": in_names,
              "out_names": out_names, "out_avals": out_avals}
    _CACHE["runner"] = runner
    return runner


def kernel(x, wq, bq, wk, bk, wv, bv, wo, bo, attn_mask):
    x = np.asarray(x, np.float32)
    in_maps = _make_in_maps(
        x, np.asarray(wq, np.float32), np.asarray(bq, np.float32),
        np.asarray(wk, np.float32), np.asarray(bk, np.float32),
        np.asarray(wv, np.float32), np.asarray(bv, np.float32),
        np.asarray(wo, np.float32))
    parts = _get_runner()["run"](in_maps)  # [8, T, D] (bf16 partials)
    parts = np.asarray(parts).astype(np.float32)
    out = parts.reshape(B, G, T, D).sum(axis=1) + np.asarray(bo, np.float32)
    return out.astype(np.float32)
